# revision 16
# baseline (speedup 1.0000x reference)
"""GAT layer (4-head, 256->4x64) on 8 Trainium2 NeuronCores.

Strategy (1D node partitioning per the sharding hint):
  - core c owns dst nodes [c*6250, (c+1)*6250) and every edge whose dst
    falls in that range (host buckets edges; sorts by dst).
  - phase A: each core projects its own node block: xh_ext = x_c @ [W | W@att_src | W@att_dst]
    producing bf16 table rows [xh bf16 (256) | a_src f32 (as 8 bf16) |
    a_dst f32 (as 8 bf16) | pad] of 768B, plus an SBUF copy of local a_dst.
  - phase B: AllGather of the table (the "halo exchange" -- with a random
    graph every core needs nearly all rows).
  - phase C: per 128-dst window: dma_gather of the window's edges' rows by
    src id (two calls: table halves, since dma_gather indices are int16),
    one-hot matmul scatter-add into a PSUM window accumulator of
    [sum_e ex_e*xh_e | sum_e ex_e]; normalize after accumulation (softmax
    denominator division commutes with the sum). Segment max is skipped:
    logits are bounded (|alpha| <~ 20) so raw exp is safe in fp32, and the
    reference's +1e-16 epsilon is preserved.
  - per-edge attention: att = ex * (1/denom)[dst], with dst-indexed gathers
    done as one-hot matmuls (lhsT = one-hot^T); a_dst and 1/denom are fed
    as bf16 hi/lo pairs so those gathers stay fp32-exact.
  - att output is produced in sorted-edge order; the host applies the
    inverse permutation.
"""

import os
import sys
import types

if os.environ.get("JAX_PLATFORMS") == "cpu":
    # the PJRT execute path needs the neuron/axon backend
    del os.environ["JAX_PLATFORMS"]

import numpy as np

import concourse.bass as bass
import concourse.mybir as mybir
import concourse.tile as tile
from concourse import bacc
from concourse.bass_utils import run_bass_kernel_spmd
from concourse.library_config import mlp

N = 50000
E = 800000
IN = 256
H = 4
C = 64
NEG_SLOPE = 0.2
EPS = 1e-16
NCORES = 8
P = 128
NPC = N // NCORES          # nodes per core
NW = (NPC + P - 1) // P    # dst windows per core
D_XH = H * C               # 256
D_EXT = D_XH + H           # 260 : xh | ex  (scatter-matmul rhs width)
TW = 384                   # bf16 table row: xh(256) | a_src f32(8) | a_dst f32(8) | pad
PAD_DST = 999.0

F32 = mybir.dt.float32
BF16 = mybir.dt.bfloat16
I16 = mybir.dt.int16
I32 = mybir.dt.int32


def _install_ntff_hook_shim():
    try:
        import antenv.axon_hooks  # noqa: F401
        return
    except ImportError:
        pass
    try:
        from trn_agent_boot.trn_boot import _ntff_profile_via_ctypes

        so = "/opt/axon/libaxon_pjrt.so"
        if not os.path.exists(so):
            return
        hook = _ntff_profile_via_ctypes(so)
        mod = types.ModuleType("antenv.axon_hooks")
        mod.get_axon_ntff_profile_hook = lambda: hook
        import antenv

        antenv.axon_hooks = mod
        sys.modules["antenv.axon_hooks"] = mod
    except Exception:
        pass


_install_ntff_hook_shim()


def _wrap16(vals):
    """int16 index list (len n, n%128==0) -> [128, n//16] dma_gather layout:
    logical index i sits at [i % 16, i // 16], replicated over the 8
    16-partition Q7 groups."""
    n = len(vals)
    arr = np.zeros((16, n // 16), np.int16)
    arr[np.arange(n) % 16, np.arange(n) // 16] = vals
    return np.tile(arr, (8, 1))


# ---------------------------------------------------------------- host prep
def _prep(x, edge_index, W, att_src, att_dst, n=N, ncores=NCORES):
    npc = n // ncores
    nw = (npc + P - 1) // P
    half = n // 2
    x = np.asarray(x, np.float32)
    W = np.asarray(W, np.float32)
    att_src = np.asarray(att_src, np.float32)
    att_dst = np.asarray(att_dst, np.float32)
    src = np.asarray(edge_index[0], np.int64)
    dst = np.asarray(edge_index[1], np.int64)

    W3 = W.reshape(IN, H, C)
    w_src = np.einsum("khc,hc->kh", W3.astype(np.float64), att_src.astype(np.float64))
    w_dst = np.einsum("khc,hc->kh", W3.astype(np.float64), att_dst.astype(np.float64))
    wext = np.concatenate(
        [W, w_src.astype(np.float32), w_dst.astype(np.float32)], axis=1
    )  # [IN, 264]

    # table-row permutation: quarter-major so each pipelined AllGather
    # quarter writes a contiguous output block. bounds align to 128-row tiles.
    ntiles = (npc + P - 1) // P
    qt = sorted(
        {0, npc}
        | {min((ntiles * q) // 4, ntiles - 1) * P for q in (1, 2, 3)}
    )
    qt = [v for v in qt if v <= npc]
    qb = np.array(qt, np.int64)

    def perm_row(nodes):
        nodes = np.asarray(nodes, np.int64)
        r = nodes // npc
        i = nodes % npc
        q = np.searchsorted(qb, i, side="right") - 1
        qlen = qb[q + 1] - qb[q]
        return ncores * qb[q] + r * qlen + (i - qb[q])

    owner = dst // npc
    # per (core, window): edge lists split into lo (src < half) / hi
    lists = [[None] * nw for _ in range(ncores)]
    for c in range(ncores):
        ids = np.nonzero(owner == c)[0]
        dl = (dst[ids] - c * npc).astype(np.int64)
        order = np.argsort(dl, kind="stable")
        ids = ids[order]
        dl = dl[order]
        w_of = dl // P
        for w in range(nw):
            sel = w_of == w
            eids = ids[sel]
            edl = dl[sel]
            prow = perm_row(src[eids])
            lo = prow < half
            lists[c][w] = (
                (eids[lo], edl[lo], prow[lo]),
                (eids[~lo], edl[~lo], prow[~lo] - half),
            )

    kl = [
        max(1, max(-(-len(lists[c][w][0][0]) // P) for c in range(ncores)))
        for w in range(nw)
    ]
    kh = [
        max(1, max(-(-len(lists[c][w][1][0]) // P) for c in range(ncores)))
        for w in range(nw)
    ]
    kw = [kl[w] + kh[w] for w in range(nw)]
    ctot = int(sum(kw))
    col0 = np.cumsum([0] + kw)

    idx16 = np.zeros((ncores, P, 8 * ctot), np.int16)
    dstrel = np.full((ncores, P, ctot), PAD_DST, np.float32)
    att_pos = []
    for c in range(ncores):
        pos_all = []
        ids_all = []
        for w in range(nw):
            (eL, dL, sL), (eH, dH, sH) = lists[c][w]
            for (ee, dd, ss, kk, cbase) in (
                (eL, dL, sL, kl[w], col0[w]),
                (eH, dH, sH, kh[w], col0[w] + kl[w]),
            ):
                npad = kk * P
                svals = np.zeros(npad, np.int64)
                svals[: len(ss)] = ss
                idx16[c, :, 8 * cbase : 8 * (cbase + kk)] = _wrap16(svals)
                pp = cbase * P + np.arange(len(ee))
                pos_all.append(pp)
                ids_all.append(ee)
                dstrel[c, pp % P, pp // P] = (dd - w * P).astype(np.float32)
        att_pos.append((np.concatenate(ids_all), np.concatenate(pos_all)))

    iota_col = np.arange(P, dtype=np.float32).astype(np.float32)
    in_maps = []
    for c in range(ncores):
        xt = np.ascontiguousarray(x[c * npc : (c + 1) * npc].T)
        in_maps.append(
            {
                "xt": xt,
                "wext": wext,
                "idx16": idx16[c],
                "dstrel_h": dstrel[c].astype(np.dtype("bfloat16"))
                if hasattr(np, "bfloat16")
                else dstrel[c],  # replaced below
                "identity_h": None,
            }
        )
    # bf16 via ml_dtypes
    import ml_dtypes

    bf = ml_dtypes.bfloat16
    iota_rep = np.broadcast_to(np.arange(P, dtype=np.float32), (P, P))
    identity = np.eye(P, dtype=np.float32)
    for c in range(ncores):
        in_maps[c]["dstrel_h"] = dstrel[c].astype(bf)
        in_maps[c]["dstrel32"] = dstrel[c]
        in_maps[c]["iota_col32"] = iota_col.reshape(P, 1)
        in_maps[c]["iota_rep32"] = np.ascontiguousarray(iota_rep)
        in_maps[c]["identity_h"] = identity.astype(bf)

    meta = {
        "qt": tuple(int(v) for v in qt),
        "n": n,
        "ncores": ncores,
        "npc": npc,
        "nw": nw,
        "kl": tuple(kl),
        "kh": tuple(kh),
        "kw": tuple(kw),
        "ctot": ctot,
        "col0": tuple(int(v) for v in col0),
    }
    return in_maps, meta, att_pos


# ---------------------------------------------------------------- kernel gen
def _build(meta, linearize=False):
    npc = meta["npc"]
    nw = meta["nw"]
    kl, kh, kw = meta["kl"], meta["kh"], meta["kw"]
    ctot = meta["ctot"]
    col0 = meta["col0"]
    ncores = meta["ncores"]
    n = meta["n"]
    half = n // 2
    maxk = max(kw)
    D_AH = 264  # phase-A psum width: xh | a_src | a_dst

    nc = bacc.Bacc(
        "TRN2", target_bir_lowering=False, debug=False, num_devices=ncores
    )

    xt = nc.declare_dram_parameter("xt", [IN, npc], F32, isOutput=False)
    wext = nc.declare_dram_parameter("wext", [IN, D_AH], F32, isOutput=False)
    idx16_d = nc.declare_dram_parameter("idx16", [P, 8 * ctot], I16, isOutput=False)
    dstrel_d = nc.declare_dram_parameter("dstrel_h", [P, ctot], BF16, isOutput=False)
    dstrel32_d = nc.declare_dram_parameter("dstrel32", [P, ctot], F32, isOutput=False)
    iota_col_d = nc.declare_dram_parameter("iota_col32", [P, 1], F32, isOutput=False)
    iota_rep_d = nc.declare_dram_parameter("iota_rep32", [P, P], F32, isOutput=False)
    identity_d = nc.declare_dram_parameter("identity_h", [P, P], BF16, isOutput=False)

    out_d = nc.declare_dram_parameter("out", [npc, D_XH], F32, isOutput=True)
    att_d = nc.declare_dram_parameter("att", [ctot * P, H], F32, isOutput=True)

    xh_local = nc.dram_tensor("xh_local", [npc, TW], BF16)
    xh_table = nc.dram_tensor("xh_table", [n, TW], BF16, addr_space="Shared")

    a_dst_sb = nc.alloc_sbuf_tensor("a_dst_sb", [P, H * nw], F32)
    idx16_sb = nc.alloc_sbuf_tensor("idx16_sb", [P, 8 * ctot], I16)
    dstrel_sb = nc.alloc_sbuf_tensor("dstrel_sb", [P, ctot], BF16)
    dstrel32_sb = nc.alloc_sbuf_tensor("dstrel32_sb", [P, ctot], F32)
    iota_col_sb = nc.alloc_sbuf_tensor("iota_col_sb", [P, 1], F32)
    iota_rep_sb = nc.alloc_sbuf_tensor("iota_rep_sb", [P, P], F32)
    identity_sb = nc.alloc_sbuf_tensor("identity_sb", [P, P], BF16)
    adhl_sb = nc.alloc_sbuf_tensor("adhl_sb", [P, nw * 2 * H], BF16)

    ntiles = (npc + P - 1) // P
    qt = meta["qt"]

    # Q7 library with the dma_gather kernel; must precede any dma_gather.
    nc.gpsimd.load_library(mlp)
    from concourse.tile import add_dep_helper

    # ---------------- phase A ------------------------------------------
    with tile.TileContext(nc, linearize=linearize) as tc:
        with (
            tc.tile_pool(name="xa", bufs=1) as xa,
            tc.tile_pool(name="wa", bufs=1) as wa,
            tc.tile_pool(name="sba", bufs=3) as sba,
            tc.tile_pool(name="psa", bufs=2, space="PSUM") as psa,
        ):
            xt_sb = [
                xa.tile([P, npc], F32, tag=f"xt{i}", name=f"xt_sb{i}")
                for i in range(2)
            ]
            w_sb = [
                wa.tile([P, D_AH], F32, tag=f"w{i}", name=f"w_sb{i}")
                for i in range(2)
            ]
            for i in range(2):
                nc.sync.dma_start(out=xt_sb[i][:], in_=xt[i * P : (i + 1) * P, :])
                nc.sync.dma_start(out=w_sb[i][:], in_=wext[i * P : (i + 1) * P, :])
            nc.sync.dma_start(out=idx16_sb[:, :], in_=idx16_d[:, :])
            nc.sync.dma_start(out=dstrel_sb[:, :], in_=dstrel_d[:, :])
            nc.sync.dma_start(out=dstrel32_sb[:, :], in_=dstrel32_d[:, :])
            nc.sync.dma_start(out=iota_col_sb[:, :], in_=iota_col_d[:, :])
            nc.sync.dma_start(out=iota_rep_sb[:, :], in_=iota_rep_d[:, :])
            nc.sync.dma_start(out=identity_sb[:, :], in_=identity_d[:, :])

            nc.vector.memset(a_dst_sb[:, :], 0)
            row_dmas = []
            for t in range(ntiles):
                rows = min(P, npc - t * P)
                ps = psa.tile([P, D_AH], F32, space="PSUM", tag="psA")
                nc.tensor.matmul(
                    out=ps[:rows, :],
                    lhsT=xt_sb[0][:, t * P : t * P + rows],
                    rhs=w_sb[0][:],
                    start=True,
                    stop=False,
                )
                nc.tensor.matmul(
                    out=ps[:rows, :],
                    lhsT=xt_sb[1][:, t * P : t * P + rows],
                    rhs=w_sb[1][:],
                    start=False,
                    stop=True,
                )
                row = sba.tile([P, TW], BF16, tag="rowA")
                nc.vector.tensor_copy(
                    out=row[:rows, 0:D_XH], in_=ps[:rows, 0:D_XH]
                )
                # a_src, a_dst kept exact: f32 bits stored in bf16 columns
                nc.vector.tensor_copy(
                    out=row[:rows, D_XH : D_XH + 16].bitcast(F32),
                    in_=ps[:rows, D_XH:D_AH],
                )
                nc.vector.tensor_copy(
                    out=a_dst_sb[:rows, H * t : H * t + H],
                    in_=ps[:rows, D_XH + H : D_AH],
                )
                row_dmas.append(
                    nc.sync.dma_start(
                        out=xh_local[t * P : t * P + rows, :], in_=row[:rows, :]
                    )
                )
                qhit = [qi for qi in range(len(qt) - 1) if qt[qi + 1] == t * P + rows]
                if qhit:
                    qi = qhit[0]
                    r0, r1 = qt[qi], qt[qi + 1]
                    cc = nc.gpsimd.collective_compute(
                        "AllGather",
                        mybir.AluOpType.bypass,
                        replica_groups=[list(range(ncores))],
                        ins=[xh_local[r0:r1, :].opt()],
                        outs=[xh_table[ncores * r0 : ncores * r1, :].opt()],
                    )
                    for dm in row_dmas:
                        add_dep_helper(
                            cc.ins, dm.ins, sync=True,
                            reason="AllGather quarter waits on table-row DMAs",
                        )
                    row_dmas = []
            # a_dst bf16 hi/lo pairs for exact bf16-matmul gathers
            ahi32 = sba.tile([P, H * nw], F32, tag="ahi32", name="ahi32")
            adhl = adhl_sb[:, :].rearrange("p (w u h) -> p w u h", u=2, h=H)
            nc.vector.tensor_copy(
                out=adhl[:, :, 0, :],
                in_=a_dst_sb[:, :].rearrange("p (w h) -> p w h", h=H),
            )
            nc.vector.tensor_copy(
                out=ahi32[:].rearrange("p (w h) -> p w h", h=H), in_=adhl[:, :, 0, :]
            )
            nc.vector.tensor_sub(out=ahi32[:], in0=a_dst_sb[:, :], in1=ahi32[:])
            nc.vector.tensor_copy(
                out=adhl[:, :, 1, :],
                in_=ahi32[:].rearrange("p (w h) -> p w h", h=H),
            )

    # ---------------- phase C ------------------------------------------
    t_lo = xh_table[0:half, :]
    t_hi = xh_table[half:n, :]
    with tile.TileContext(nc, linearize=linearize) as tc:
        with (
            tc.tile_pool(name="gat", bufs=3) as gp,
            tc.tile_pool(name="rhs", bufs=2) as rp,
            tc.tile_pool(name="amat", bufs=maxk + 2) as ap_,
            tc.tile_pool(name="atmat", bufs=2 * ((maxk + 3) // 4) + 2) as atp,
            tc.tile_pool(name="small", bufs=3) as sp,
            tc.tile_pool(name="winps", bufs=2, space="PSUM") as wpp,
            tc.tile_pool(name="dstps", bufs=2, space="PSUM") as dpp,
            tc.tile_pool(name="smps", bufs=2, space="PSUM") as spp,
            tc.tile_pool(name="exw", bufs=2) as exp_,
            tc.tile_pool(name="outw", bufs=2) as op_,
        ):
            for w in range(nw):
                k = kw[w]
                g0 = col0[w]
                rows = min(P, npc - w * P)
                gatb = gp.tile([P, k, TW], BF16, tag="gat")
                # dma_gather crashes the device above ~1024 idxs per call
                for base, cnt, tbl in (
                    (0, kl[w], t_lo),
                    (kl[w], kh[w], t_hi),
                ):
                    off = 0
                    while off < cnt:
                        m = min(8, cnt - off)
                        c0 = g0 + base + off
                        nc.gpsimd.dma_gather(
                            gatb[:, base + off : base + off + m, :],
                            tbl,
                            idx16_sb[:, 8 * c0 : 8 * (c0 + m)],
                            m * P,
                            m * P,
                            TW,
                        )
                        off += m
                # one-hot builds + transposes + a_dst gather matmuls
                amats = []
                atbs = []
                adstb = spp.tile([P, k, 2 * H], F32, space="PSUM", tag="smps")
                for j in range(k):
                    g = g0 + j
                    jm = j % 4
                    if jm == 0:
                        nb = min(4, k - j)
                        dstTb = dpp.tile([P, 4 * P], BF16, space="PSUM", tag="dstT")
                        atb = atp.tile([P, 4 * P], BF16, tag="at")
                    nc.tensor.transpose(
                        out=dstTb[:, jm * P : (jm + 1) * P],
                        in_=dstrel_sb[:, g : g + 1].to_broadcast([P, P]),
                        identity=identity_sb[:],
                    )
                    if jm == nb - 1:
                        nc.vector.tensor_tensor(
                            out=atb[:, 0 : nb * P],
                            in0=iota_col_sb[:, 0:1].to_broadcast([P, nb * P]),
                            in1=dstTb[:, 0 : nb * P],
                            op=mybir.AluOpType.is_equal,
                        )
                        atbs.append(atb)
                    amat = ap_.tile([P, P], BF16, tag="amat")
                    nc.vector.tensor_tensor(
                        out=amat[:],
                        in0=dstrel32_sb[:, g : g + 1].to_broadcast([P, P]),
                        in1=iota_rep_sb[:],
                        op=mybir.AluOpType.is_equal,
                    )
                    amats.append(amat)
                for j in range(k):
                    nc.tensor.matmul(
                        out=adstb[:, j, :],
                        lhsT=atbs[j // 4][:, (j % 4) * P : (j % 4 + 1) * P],
                        rhs=adhl_sb[:, 2 * H * w : 2 * H * (w + 1)],
                        start=True,
                        stop=True,
                    )
                # batched alpha -> ex -> rhs
                exw = exp_.tile([P, k, H], F32, tag="exw")
                asrc = gatb[:, :, D_XH : D_XH + 8].bitcast(F32)  # [P,k,4]
                nc.vector.tensor_add(
                    out=exw[:], in0=asrc, in1=adstb[:, :, 0:H]
                )
                nc.vector.tensor_add(out=exw[:], in0=exw[:], in1=adstb[:, :, H:])
                lk = sp.tile([P, k, H], F32, tag="lk")
                nc.vector.tensor_scalar_mul(out=lk[:], in0=exw[:], scalar1=NEG_SLOPE)
                nc.vector.tensor_tensor(
                    out=exw[:], in0=exw[:], in1=lk[:], op=mybir.AluOpType.max
                )
                nc.scalar.activation(
                    out=exw[:], in_=exw[:], func=mybir.ActivationFunctionType.Exp
                )
                rhsb = rp.tile([P, k, D_EXT], BF16, tag="rhs")
                nc.vector.tensor_copy(out=rhsb[:, :, D_XH:D_EXT], in_=exw[:])
                nc.vector.tensor_tensor(
                    out=rhsb[:, :, 0:D_XH].rearrange("p k (h c) -> p k h c", h=H),
                    in0=gatb[:, :, 0:D_XH].rearrange("p k (h c) -> p k h c", h=H),
                    in1=exw[:].to_broadcast([P, k, H, C]),
                    op=mybir.AluOpType.mult,
                )
                # scatter-accumulate
                ps = wpp.tile([P, D_EXT], F32, space="PSUM", tag="win")
                for j in range(k):
                    nc.tensor.matmul(
                        out=ps[:],
                        lhsT=amats[j][:],
                        rhs=rhsb[:, j, :],
                        start=(j == 0),
                        stop=(j == k - 1),
                    )
                # normalize
                den = sp.tile([P, H], F32, tag="den")
                nc.vector.tensor_scalar_add(
                    out=den[:], in0=ps[:, D_XH:D_EXT], scalar1=EPS
                )
                denrec = sp.tile([P, H], F32, tag="denrec")
                nc.vector.reciprocal(out=denrec[:], in_=den[:])
                outsb = op_.tile([P, D_XH], F32, tag="outw")
                nc.vector.tensor_tensor(
                    out=outsb[:].rearrange("p (h c) -> p h c", h=H),
                    in0=ps[:, 0:D_XH].rearrange("p (h c) -> p h c", h=H),
                    in1=denrec[:].to_broadcast([P, H, C]),
                    op=mybir.AluOpType.mult,
                )
                nc.sync.dma_start(
                    out=out_d[w * P : w * P + rows, :], in_=outsb[:rows, :]
                )
                # denrec hi/lo for exact per-edge gather
                drhl = sp.tile([P, 2 * H], BF16, tag="drhl")
                dr32 = sp.tile([P, H], F32, tag="dr32")
                nc.vector.tensor_copy(out=drhl[:, 0:H], in_=denrec[:])
                nc.vector.tensor_copy(out=dr32[:], in_=drhl[:, 0:H])
                nc.vector.tensor_sub(out=dr32[:], in0=denrec[:], in1=dr32[:])
                nc.vector.tensor_copy(out=drhl[:, H:], in_=dr32[:])
                attb = spp.tile([P, k, 2 * H], F32, space="PSUM", tag="smps")
                for j in range(k):
                    nc.tensor.matmul(
                        out=attb[:, j, :],
                        lhsT=atbs[j // 4][:, (j % 4) * P : (j % 4 + 1) * P],
                        rhs=drhl[:],
                        start=True,
                        stop=True,
                    )
                attw = exp_.tile([P, k, H], F32, tag="attw")
                nc.vector.tensor_copy(out=attw[:], in_=attb[:, :, 0:H])
                nc.vector.tensor_add(out=attw[:], in0=attw[:], in1=attb[:, :, H:])
                nc.vector.tensor_tensor(
                    out=attw[:], in0=attw[:], in1=exw[:], op=mybir.AluOpType.mult
                )
                att_view = att_d[g0 * P : (g0 + k) * P, :].rearrange(
                    "(k p) h -> p k h", p=P
                )
                nc.sync.dma_start(out=att_view, in_=attw[:])

    nc.compile()
    return nc


_CACHE = {}


def _get_kernel(meta):
    key = (meta["n"], meta["ncores"], meta["kl"], meta["kh"])
    if key not in _CACHE:
        _CACHE[key] = _build(meta)
    return _CACHE[key]


def kernel(x, edge_index, W, att_src, att_dst):
    in_maps, meta, att_pos = _prep(x, edge_index, W, att_src, att_dst)
    nc = _get_kernel(meta)
    res = run_bass_kernel_spmd(nc, in_maps, core_ids=list(range(meta["ncores"])))
    ncores = meta["ncores"]
    out = np.concatenate([res.results[c]["out"] for c in range(ncores)], axis=0)
    att = np.empty((E, H), np.float32)
    for c in range(ncores):
        ids, pos = att_pos[c]
        att[ids] = res.results[c]["att"][pos]
    return out, att


# revision 17
# speedup vs baseline: 1.0030x; 1.0030x over previous
"""GAT layer (4-head, 256->4x64) on 8 Trainium2 NeuronCores.

Strategy (1D node partitioning per the sharding hint):
  - core c owns dst nodes [c*6250, (c+1)*6250) and every edge whose dst
    falls in that range (host buckets edges; sorts by dst).
  - phase A: each core projects its own node block: xh_ext = x_c @ [W | W@att_src | W@att_dst]
    producing bf16 table rows [xh bf16 (256) | a_src f32 (as 8 bf16) |
    a_dst f32 (as 8 bf16) | pad] of 768B, plus an SBUF copy of local a_dst.
  - phase B: AllGather of the table (the "halo exchange" -- with a random
    graph every core needs nearly all rows).
  - phase C: per 128-dst window: dma_gather of the window's edges' rows by
    src id (two calls: table halves, since dma_gather indices are int16),
    one-hot matmul scatter-add into a PSUM window accumulator of
    [sum_e ex_e*xh_e | sum_e ex_e]; normalize after accumulation (softmax
    denominator division commutes with the sum). Segment max is skipped:
    logits are bounded (|alpha| <~ 20) so raw exp is safe in fp32, and the
    reference's +1e-16 epsilon is preserved.
  - per-edge attention: att = ex * (1/denom)[dst], with dst-indexed gathers
    done as one-hot matmuls (lhsT = one-hot^T); a_dst and 1/denom are fed
    as bf16 hi/lo pairs so those gathers stay fp32-exact.
  - att output is produced in sorted-edge order; the host applies the
    inverse permutation.
"""

import os
import sys
import types

if os.environ.get("JAX_PLATFORMS") == "cpu":
    # the PJRT execute path needs the neuron/axon backend
    del os.environ["JAX_PLATFORMS"]

import numpy as np

import concourse.bass as bass
import concourse.mybir as mybir
import concourse.tile as tile
from concourse import bacc
from concourse.bass_utils import run_bass_kernel_spmd
from concourse.library_config import mlp

N = 50000
E = 800000
IN = 256
H = 4
C = 64
NEG_SLOPE = 0.2
EPS = 1e-16
NCORES = 8
P = 128
NPC = N // NCORES          # nodes per core
NW = (NPC + P - 1) // P    # dst windows per core
D_XH = H * C               # 256
D_EXT = D_XH + H           # 260 : xh | ex  (scatter-matmul rhs width)
TW = 384                   # bf16 table row: xh(256) | a_src f32(8) | a_dst f32(8) | pad
PAD_DST = 999.0

F32 = mybir.dt.float32
BF16 = mybir.dt.bfloat16
I16 = mybir.dt.int16
I32 = mybir.dt.int32


def _install_ntff_hook_shim():
    try:
        import antenv.axon_hooks  # noqa: F401
        return
    except ImportError:
        pass
    try:
        from trn_agent_boot.trn_boot import _ntff_profile_via_ctypes

        so = "/opt/axon/libaxon_pjrt.so"
        if not os.path.exists(so):
            return
        hook = _ntff_profile_via_ctypes(so)
        mod = types.ModuleType("antenv.axon_hooks")
        mod.get_axon_ntff_profile_hook = lambda: hook
        import antenv

        antenv.axon_hooks = mod
        sys.modules["antenv.axon_hooks"] = mod
    except Exception:
        pass


_install_ntff_hook_shim()


def _wrap16(vals):
    """int16 index list (len n, n%128==0) -> [128, n//16] dma_gather layout:
    logical index i sits at [i % 16, i // 16], replicated over the 8
    16-partition Q7 groups."""
    n = len(vals)
    arr = np.zeros((16, n // 16), np.int16)
    arr[np.arange(n) % 16, np.arange(n) // 16] = vals
    return np.tile(arr, (8, 1))


# ---------------------------------------------------------------- host prep
def _prep(x, edge_index, W, att_src, att_dst, n=N, ncores=NCORES):
    npc = n // ncores
    nw = (npc + P - 1) // P
    half = n // 2
    x = np.asarray(x, np.float32)
    W = np.asarray(W, np.float32)
    att_src = np.asarray(att_src, np.float32)
    att_dst = np.asarray(att_dst, np.float32)
    src = np.asarray(edge_index[0], np.int64)
    dst = np.asarray(edge_index[1], np.int64)

    W3 = W.reshape(IN, H, C)
    w_src = np.einsum("khc,hc->kh", W3.astype(np.float64), att_src.astype(np.float64))
    w_dst = np.einsum("khc,hc->kh", W3.astype(np.float64), att_dst.astype(np.float64))
    wext = np.concatenate(
        [W, w_src.astype(np.float32), w_dst.astype(np.float32)], axis=1
    )  # [IN, 264]

    # table-row permutation: quarter-major so each pipelined AllGather
    # quarter writes a contiguous output block. bounds align to 128-row tiles.
    ntiles = (npc + P - 1) // P
    # single gather quarter measured faster than 4 pipelined ones
    qt = [0, npc]
    qb = np.array(qt, np.int64)

    def perm_row(nodes):
        nodes = np.asarray(nodes, np.int64)
        r = nodes // npc
        i = nodes % npc
        q = np.searchsorted(qb, i, side="right") - 1
        qlen = qb[q + 1] - qb[q]
        return ncores * qb[q] + r * qlen + (i - qb[q])

    owner = dst // npc
    # per (core, window): edge lists split into lo (src < half) / hi
    lists = [[None] * nw for _ in range(ncores)]
    for c in range(ncores):
        ids = np.nonzero(owner == c)[0]
        dl = (dst[ids] - c * npc).astype(np.int64)
        order = np.argsort(dl, kind="stable")
        ids = ids[order]
        dl = dl[order]
        w_of = dl // P
        for w in range(nw):
            sel = w_of == w
            eids = ids[sel]
            edl = dl[sel]
            prow = perm_row(src[eids])
            lo = prow < half
            lists[c][w] = (
                (eids[lo], edl[lo], prow[lo]),
                (eids[~lo], edl[~lo], prow[~lo] - half),
            )

    kl = [
        max(1, max(-(-len(lists[c][w][0][0]) // P) for c in range(ncores)))
        for w in range(nw)
    ]
    kh = [
        max(1, max(-(-len(lists[c][w][1][0]) // P) for c in range(ncores)))
        for w in range(nw)
    ]
    kw = [kl[w] + kh[w] for w in range(nw)]
    ctot = int(sum(kw))
    col0 = np.cumsum([0] + kw)

    idx16 = np.zeros((ncores, P, 8 * ctot), np.int16)
    dstrel = np.full((ncores, P, ctot), PAD_DST, np.float32)
    att_pos = []
    for c in range(ncores):
        pos_all = []
        ids_all = []
        for w in range(nw):
            (eL, dL, sL), (eH, dH, sH) = lists[c][w]
            for (ee, dd, ss, kk, cbase) in (
                (eL, dL, sL, kl[w], col0[w]),
                (eH, dH, sH, kh[w], col0[w] + kl[w]),
            ):
                npad = kk * P
                svals = np.zeros(npad, np.int64)
                svals[: len(ss)] = ss
                idx16[c, :, 8 * cbase : 8 * (cbase + kk)] = _wrap16(svals)
                pp = cbase * P + np.arange(len(ee))
                pos_all.append(pp)
                ids_all.append(ee)
                dstrel[c, pp % P, pp // P] = (dd - w * P).astype(np.float32)
        att_pos.append((np.concatenate(ids_all), np.concatenate(pos_all)))

    iota_col = np.arange(P, dtype=np.float32).astype(np.float32)
    in_maps = []
    for c in range(ncores):
        xt = np.ascontiguousarray(x[c * npc : (c + 1) * npc].T)
        in_maps.append(
            {
                "xt": xt,
                "wext": wext,
                "idx16": idx16[c],
                "dstrel_h": dstrel[c].astype(np.dtype("bfloat16"))
                if hasattr(np, "bfloat16")
                else dstrel[c],  # replaced below
                "identity_h": None,
            }
        )
    # bf16 via ml_dtypes
    import ml_dtypes

    bf = ml_dtypes.bfloat16
    iota_rep = np.broadcast_to(np.arange(P, dtype=np.float32), (P, P))
    identity = np.eye(P, dtype=np.float32)
    for c in range(ncores):
        in_maps[c]["dstrel_h"] = dstrel[c].astype(bf)
        in_maps[c]["dstrel32"] = dstrel[c]
        in_maps[c]["iota_col32"] = iota_col.reshape(P, 1)
        in_maps[c]["iota_rep32"] = np.ascontiguousarray(iota_rep)
        in_maps[c]["identity_h"] = identity.astype(bf)

    meta = {
        "qt": tuple(int(v) for v in qt),
        "n": n,
        "ncores": ncores,
        "npc": npc,
        "nw": nw,
        "kl": tuple(kl),
        "kh": tuple(kh),
        "kw": tuple(kw),
        "ctot": ctot,
        "col0": tuple(int(v) for v in col0),
    }
    return in_maps, meta, att_pos


# ---------------------------------------------------------------- kernel gen
def _build(meta, linearize=False):
    npc = meta["npc"]
    nw = meta["nw"]
    kl, kh, kw = meta["kl"], meta["kh"], meta["kw"]
    ctot = meta["ctot"]
    col0 = meta["col0"]
    ncores = meta["ncores"]
    n = meta["n"]
    half = n // 2
    maxk = max(kw)
    D_AH = 264  # phase-A psum width: xh | a_src | a_dst

    nc = bacc.Bacc(
        "TRN2", target_bir_lowering=False, debug=False, num_devices=ncores
    )

    xt = nc.declare_dram_parameter("xt", [IN, npc], F32, isOutput=False)
    wext = nc.declare_dram_parameter("wext", [IN, D_AH], F32, isOutput=False)
    idx16_d = nc.declare_dram_parameter("idx16", [P, 8 * ctot], I16, isOutput=False)
    dstrel_d = nc.declare_dram_parameter("dstrel_h", [P, ctot], BF16, isOutput=False)
    dstrel32_d = nc.declare_dram_parameter("dstrel32", [P, ctot], F32, isOutput=False)
    iota_col_d = nc.declare_dram_parameter("iota_col32", [P, 1], F32, isOutput=False)
    iota_rep_d = nc.declare_dram_parameter("iota_rep32", [P, P], F32, isOutput=False)
    identity_d = nc.declare_dram_parameter("identity_h", [P, P], BF16, isOutput=False)

    out_d = nc.declare_dram_parameter("out", [npc, D_XH], F32, isOutput=True)
    att_d = nc.declare_dram_parameter("att", [ctot * P, H], F32, isOutput=True)

    xh_local = nc.dram_tensor("xh_local", [npc, TW], BF16)
    xh_table = nc.dram_tensor("xh_table", [n, TW], BF16, addr_space="Shared")

    a_dst_sb = nc.alloc_sbuf_tensor("a_dst_sb", [P, H * nw], F32)
    idx16_sb = nc.alloc_sbuf_tensor("idx16_sb", [P, 8 * ctot], I16)
    dstrel_sb = nc.alloc_sbuf_tensor("dstrel_sb", [P, ctot], BF16)
    dstrel32_sb = nc.alloc_sbuf_tensor("dstrel32_sb", [P, ctot], F32)
    iota_col_sb = nc.alloc_sbuf_tensor("iota_col_sb", [P, 1], F32)
    iota_rep_sb = nc.alloc_sbuf_tensor("iota_rep_sb", [P, P], F32)
    identity_sb = nc.alloc_sbuf_tensor("identity_sb", [P, P], BF16)
    adhl_sb = nc.alloc_sbuf_tensor("adhl_sb", [P, nw * 2 * H], BF16)

    ntiles = (npc + P - 1) // P
    qt = meta["qt"]

    # Q7 library with the dma_gather kernel; must precede any dma_gather.
    nc.gpsimd.load_library(mlp)
    from concourse.tile import add_dep_helper

    # ---------------- phase A ------------------------------------------
    with tile.TileContext(nc, linearize=linearize) as tc:
        with (
            tc.tile_pool(name="xa", bufs=1) as xa,
            tc.tile_pool(name="wa", bufs=1) as wa,
            tc.tile_pool(name="sba", bufs=3) as sba,
            tc.tile_pool(name="psa", bufs=2, space="PSUM") as psa,
        ):
            xt_sb = [
                xa.tile([P, npc], F32, tag=f"xt{i}", name=f"xt_sb{i}")
                for i in range(2)
            ]
            w_sb = [
                wa.tile([P, D_AH], F32, tag=f"w{i}", name=f"w_sb{i}")
                for i in range(2)
            ]
            for i in range(2):
                nc.sync.dma_start(out=xt_sb[i][:], in_=xt[i * P : (i + 1) * P, :])
                nc.sync.dma_start(out=w_sb[i][:], in_=wext[i * P : (i + 1) * P, :])
            nc.sync.dma_start(out=idx16_sb[:, :], in_=idx16_d[:, :])
            nc.sync.dma_start(out=dstrel_sb[:, :], in_=dstrel_d[:, :])
            nc.sync.dma_start(out=dstrel32_sb[:, :], in_=dstrel32_d[:, :])
            nc.sync.dma_start(out=iota_col_sb[:, :], in_=iota_col_d[:, :])
            nc.sync.dma_start(out=iota_rep_sb[:, :], in_=iota_rep_d[:, :])
            nc.sync.dma_start(out=identity_sb[:, :], in_=identity_d[:, :])

            nc.vector.memset(a_dst_sb[:, :], 0)
            row_dmas = []
            for t in range(ntiles):
                rows = min(P, npc - t * P)
                ps = psa.tile([P, D_AH], F32, space="PSUM", tag="psA")
                nc.tensor.matmul(
                    out=ps[:rows, :],
                    lhsT=xt_sb[0][:, t * P : t * P + rows],
                    rhs=w_sb[0][:],
                    start=True,
                    stop=False,
                )
                nc.tensor.matmul(
                    out=ps[:rows, :],
                    lhsT=xt_sb[1][:, t * P : t * P + rows],
                    rhs=w_sb[1][:],
                    start=False,
                    stop=True,
                )
                row = sba.tile([P, TW], BF16, tag="rowA")
                nc.vector.tensor_copy(
                    out=row[:rows, 0:D_XH], in_=ps[:rows, 0:D_XH]
                )
                # a_src, a_dst kept exact: f32 bits stored in bf16 columns
                nc.vector.tensor_copy(
                    out=row[:rows, D_XH : D_XH + 16].bitcast(F32),
                    in_=ps[:rows, D_XH:D_AH],
                )
                nc.vector.tensor_copy(
                    out=a_dst_sb[:rows, H * t : H * t + H],
                    in_=ps[:rows, D_XH + H : D_AH],
                )
                row_dmas.append(
                    nc.sync.dma_start(
                        out=xh_local[t * P : t * P + rows, :], in_=row[:rows, :]
                    )
                )
                qhit = [qi for qi in range(len(qt) - 1) if qt[qi + 1] == t * P + rows]
                if qhit:
                    qi = qhit[0]
                    r0, r1 = qt[qi], qt[qi + 1]
                    cc = nc.gpsimd.collective_compute(
                        "AllGather",
                        mybir.AluOpType.bypass,
                        replica_groups=[list(range(ncores))],
                        ins=[xh_local[r0:r1, :].opt()],
                        outs=[xh_table[ncores * r0 : ncores * r1, :].opt()],
                    )
                    for dm in row_dmas:
                        add_dep_helper(
                            cc.ins, dm.ins, sync=True,
                            reason="AllGather quarter waits on table-row DMAs",
                        )
                    row_dmas = []
            # a_dst bf16 hi/lo pairs for exact bf16-matmul gathers
            ahi32 = sba.tile([P, H * nw], F32, tag="ahi32", name="ahi32")
            adhl = adhl_sb[:, :].rearrange("p (w u h) -> p w u h", u=2, h=H)
            nc.vector.tensor_copy(
                out=adhl[:, :, 0, :],
                in_=a_dst_sb[:, :].rearrange("p (w h) -> p w h", h=H),
            )
            nc.vector.tensor_copy(
                out=ahi32[:].rearrange("p (w h) -> p w h", h=H), in_=adhl[:, :, 0, :]
            )
            nc.vector.tensor_sub(out=ahi32[:], in0=a_dst_sb[:, :], in1=ahi32[:])
            nc.vector.tensor_copy(
                out=adhl[:, :, 1, :],
                in_=ahi32[:].rearrange("p (w h) -> p w h", h=H),
            )

    # ---------------- phase C ------------------------------------------
    t_lo = xh_table[0:half, :]
    t_hi = xh_table[half:n, :]
    with tile.TileContext(nc, linearize=linearize) as tc:
        with (
            tc.tile_pool(name="gat", bufs=3) as gp,
            tc.tile_pool(name="rhs", bufs=2) as rp,
            tc.tile_pool(name="amat", bufs=maxk + 2) as ap_,
            tc.tile_pool(name="atmat", bufs=2 * ((maxk + 3) // 4) + 2) as atp,
            tc.tile_pool(name="small", bufs=3) as sp,
            tc.tile_pool(name="winps", bufs=2, space="PSUM") as wpp,
            tc.tile_pool(name="dstps", bufs=2, space="PSUM") as dpp,
            tc.tile_pool(name="smps", bufs=2, space="PSUM") as spp,
            tc.tile_pool(name="exw", bufs=2) as exp_,
            tc.tile_pool(name="outw", bufs=2) as op_,
        ):
            for w in range(nw):
                k = kw[w]
                g0 = col0[w]
                rows = min(P, npc - w * P)
                gatb = gp.tile([P, k, TW], BF16, tag="gat")
                # dma_gather crashes the device above ~1024 idxs per call
                for base, cnt, tbl in (
                    (0, kl[w], t_lo),
                    (kl[w], kh[w], t_hi),
                ):
                    off = 0
                    while off < cnt:
                        m = min(8, cnt - off)
                        c0 = g0 + base + off
                        nc.gpsimd.dma_gather(
                            gatb[:, base + off : base + off + m, :],
                            tbl,
                            idx16_sb[:, 8 * c0 : 8 * (c0 + m)],
                            m * P,
                            m * P,
                            TW,
                        )
                        off += m
                # one-hot builds + transposes + a_dst gather matmuls
                amats = []
                atbs = []
                adstb = spp.tile([P, k, 2 * H], F32, space="PSUM", tag="smps")
                for j in range(k):
                    g = g0 + j
                    jm = j % 4
                    if jm == 0:
                        nb = min(4, k - j)
                        dstTb = dpp.tile([P, 4 * P], BF16, space="PSUM", tag="dstT")
                        atb = atp.tile([P, 4 * P], BF16, tag="at")
                    nc.tensor.transpose(
                        out=dstTb[:, jm * P : (jm + 1) * P],
                        in_=dstrel_sb[:, g : g + 1].to_broadcast([P, P]),
                        identity=identity_sb[:],
                    )
                    if jm == nb - 1:
                        nc.vector.tensor_tensor(
                            out=atb[:, 0 : nb * P],
                            in0=iota_col_sb[:, 0:1].to_broadcast([P, nb * P]),
                            in1=dstTb[:, 0 : nb * P],
                            op=mybir.AluOpType.is_equal,
                        )
                        atbs.append(atb)
                    amat = ap_.tile([P, P], BF16, tag="amat")
                    nc.vector.tensor_tensor(
                        out=amat[:],
                        in0=dstrel32_sb[:, g : g + 1].to_broadcast([P, P]),
                        in1=iota_rep_sb[:],
                        op=mybir.AluOpType.is_equal,
                    )
                    amats.append(amat)
                for j in range(k):
                    nc.tensor.matmul(
                        out=adstb[:, j, :],
                        lhsT=atbs[j // 4][:, (j % 4) * P : (j % 4 + 1) * P],
                        rhs=adhl_sb[:, 2 * H * w : 2 * H * (w + 1)],
                        start=True,
                        stop=True,
                    )
                # batched alpha -> ex -> rhs
                exw = exp_.tile([P, k, H], F32, tag="exw")
                asrc = gatb[:, :, D_XH : D_XH + 8].bitcast(F32)  # [P,k,4]
                nc.vector.tensor_add(
                    out=exw[:], in0=asrc, in1=adstb[:, :, 0:H]
                )
                nc.vector.tensor_add(out=exw[:], in0=exw[:], in1=adstb[:, :, H:])
                lk = sp.tile([P, k, H], F32, tag="lk")
                nc.vector.tensor_scalar_mul(out=lk[:], in0=exw[:], scalar1=NEG_SLOPE)
                nc.vector.tensor_tensor(
                    out=exw[:], in0=exw[:], in1=lk[:], op=mybir.AluOpType.max
                )
                nc.scalar.activation(
                    out=exw[:], in_=exw[:], func=mybir.ActivationFunctionType.Exp
                )
                rhsb = rp.tile([P, k, D_EXT], BF16, tag="rhs")
                nc.vector.tensor_copy(out=rhsb[:, :, D_XH:D_EXT], in_=exw[:])
                nc.vector.tensor_tensor(
                    out=rhsb[:, :, 0:D_XH].rearrange("p k (h c) -> p k h c", h=H),
                    in0=gatb[:, :, 0:D_XH].rearrange("p k (h c) -> p k h c", h=H),
                    in1=exw[:].to_broadcast([P, k, H, C]),
                    op=mybir.AluOpType.mult,
                )
                # scatter-accumulate
                ps = wpp.tile([P, D_EXT], F32, space="PSUM", tag="win")
                for j in range(k):
                    nc.tensor.matmul(
                        out=ps[:],
                        lhsT=amats[j][:],
                        rhs=rhsb[:, j, :],
                        start=(j == 0),
                        stop=(j == k - 1),
                    )
                # normalize
                den = sp.tile([P, H], F32, tag="den")
                nc.vector.tensor_scalar_add(
                    out=den[:], in0=ps[:, D_XH:D_EXT], scalar1=EPS
                )
                denrec = sp.tile([P, H], F32, tag="denrec")
                nc.vector.reciprocal(out=denrec[:], in_=den[:])
                outsb = op_.tile([P, D_XH], F32, tag="outw")
                nc.vector.tensor_tensor(
                    out=outsb[:].rearrange("p (h c) -> p h c", h=H),
                    in0=ps[:, 0:D_XH].rearrange("p (h c) -> p h c", h=H),
                    in1=denrec[:].to_broadcast([P, H, C]),
                    op=mybir.AluOpType.mult,
                )
                nc.sync.dma_start(
                    out=out_d[w * P : w * P + rows, :], in_=outsb[:rows, :]
                )
                # denrec hi/lo for exact per-edge gather
                drhl = sp.tile([P, 2 * H], BF16, tag="drhl")
                dr32 = sp.tile([P, H], F32, tag="dr32")
                nc.vector.tensor_copy(out=drhl[:, 0:H], in_=denrec[:])
                nc.vector.tensor_copy(out=dr32[:], in_=drhl[:, 0:H])
                nc.vector.tensor_sub(out=dr32[:], in0=denrec[:], in1=dr32[:])
                nc.vector.tensor_copy(out=drhl[:, H:], in_=dr32[:])
                attb = spp.tile([P, k, 2 * H], F32, space="PSUM", tag="smps")
                for j in range(k):
                    nc.tensor.matmul(
                        out=attb[:, j, :],
                        lhsT=atbs[j // 4][:, (j % 4) * P : (j % 4 + 1) * P],
                        rhs=drhl[:],
                        start=True,
                        stop=True,
                    )
                attw = exp_.tile([P, k, H], F32, tag="attw")
                nc.vector.tensor_copy(out=attw[:], in_=attb[:, :, 0:H])
                nc.vector.tensor_add(out=attw[:], in0=attw[:], in1=attb[:, :, H:])
                nc.vector.tensor_tensor(
                    out=attw[:], in0=attw[:], in1=exw[:], op=mybir.AluOpType.mult
                )
                att_view = att_d[g0 * P : (g0 + k) * P, :].rearrange(
                    "(k p) h -> p k h", p=P
                )
                nc.sync.dma_start(out=att_view, in_=attw[:])

    nc.compile()
    return nc


_CACHE = {}


def _get_kernel(meta):
    key = (meta["n"], meta["ncores"], meta["kl"], meta["kh"])
    if key not in _CACHE:
        _CACHE[key] = _build(meta)
    return _CACHE[key]


def kernel(x, edge_index, W, att_src, att_dst):
    in_maps, meta, att_pos = _prep(x, edge_index, W, att_src, att_dst)
    nc = _get_kernel(meta)
    res = run_bass_kernel_spmd(nc, in_maps, core_ids=list(range(meta["ncores"])))
    ncores = meta["ncores"]
    out = np.concatenate([res.results[c]["out"] for c in range(ncores)], axis=0)
    att = np.empty((E, H), np.float32)
    for c in range(ncores):
        ids, pos = att_pos[c]
        att[ids] = res.results[c]["att"][pos]
    return out, att


# revision 18
# speedup vs baseline: 1.0870x; 1.0838x over previous
"""GAT layer (4-head, 256->4x64) on 8 Trainium2 NeuronCores.

Strategy (1D node partitioning per the sharding hint):
  - core c owns dst nodes [c*6250, (c+1)*6250) and every edge whose dst
    falls in that range (host buckets edges; sorts by dst).
  - phase A: each core projects its own node block: xh_ext = x_c @ [W | W@att_src | W@att_dst]
    producing bf16 table rows [xh bf16 (256) | a_src f32 (as 8 bf16) |
    a_dst f32 (as 8 bf16) | pad] of 768B, plus an SBUF copy of local a_dst.
  - phase B: AllGather of the table (the "halo exchange" -- with a random
    graph every core needs nearly all rows).
  - phase C: per 128-dst window: dma_gather of the window's edges' rows by
    src id (two calls: table halves, since dma_gather indices are int16),
    one-hot matmul scatter-add into a PSUM window accumulator of
    [sum_e ex_e*xh_e | sum_e ex_e]; normalize after accumulation (softmax
    denominator division commutes with the sum). Segment max is skipped:
    logits are bounded (|alpha| <~ 20) so raw exp is safe in fp32, and the
    reference's +1e-16 epsilon is preserved.
  - per-edge attention: att = ex * (1/denom)[dst], with dst-indexed gathers
    done as one-hot matmuls (lhsT = one-hot^T); a_dst and 1/denom are fed
    as bf16 hi/lo pairs so those gathers stay fp32-exact.
  - att output is produced in sorted-edge order; the host applies the
    inverse permutation.
"""

import os
import sys
import types

if os.environ.get("JAX_PLATFORMS") == "cpu":
    # the PJRT execute path needs the neuron/axon backend
    del os.environ["JAX_PLATFORMS"]

import numpy as np

import concourse.bass as bass
import concourse.mybir as mybir
import concourse.tile as tile
from concourse import bacc
from concourse.bass_utils import run_bass_kernel_spmd
from concourse.library_config import mlp

N = 50000
E = 800000
IN = 256
H = 4
C = 64
NEG_SLOPE = 0.2
EPS = 1e-16
NCORES = 8
P = 128
NPC = N // NCORES          # nodes per core
NW = (NPC + P - 1) // P    # dst windows per core
D_XH = H * C               # 256
D_EXT = D_XH + H           # 260 : xh | ex  (scatter-matmul rhs width)
TW = 384                   # bf16 table row: xh(256) | a_src f32(8) | a_dst f32(8) | pad
PAD_DST = 999.0

F32 = mybir.dt.float32
BF16 = mybir.dt.bfloat16
I16 = mybir.dt.int16
I32 = mybir.dt.int32


def _install_ntff_hook_shim():
    try:
        import antenv.axon_hooks  # noqa: F401
        return
    except ImportError:
        pass
    try:
        from trn_agent_boot.trn_boot import _ntff_profile_via_ctypes

        so = "/opt/axon/libaxon_pjrt.so"
        if not os.path.exists(so):
            return
        hook = _ntff_profile_via_ctypes(so)
        mod = types.ModuleType("antenv.axon_hooks")
        mod.get_axon_ntff_profile_hook = lambda: hook
        import antenv

        antenv.axon_hooks = mod
        sys.modules["antenv.axon_hooks"] = mod
    except Exception:
        pass


_install_ntff_hook_shim()


def _wrap16(vals):
    """int16 index list (len n, n%128==0) -> [128, n//16] dma_gather layout:
    logical index i sits at [i % 16, i // 16], replicated over the 8
    16-partition Q7 groups."""
    n = len(vals)
    arr = np.zeros((16, n // 16), np.int16)
    arr[np.arange(n) % 16, np.arange(n) // 16] = vals
    return np.tile(arr, (8, 1))


# ---------------------------------------------------------------- host prep
def _prep(x, edge_index, W, att_src, att_dst, n=N, ncores=NCORES):
    npc = n // ncores
    nw = (npc + P - 1) // P
    half = n // 2
    x = np.asarray(x, np.float32)
    W = np.asarray(W, np.float32)
    att_src = np.asarray(att_src, np.float32)
    att_dst = np.asarray(att_dst, np.float32)
    src = np.asarray(edge_index[0], np.int64)
    dst = np.asarray(edge_index[1], np.int64)

    W3 = W.reshape(IN, H, C)
    w_src = np.einsum("khc,hc->kh", W3.astype(np.float64), att_src.astype(np.float64))
    w_dst = np.einsum("khc,hc->kh", W3.astype(np.float64), att_dst.astype(np.float64))
    wext = np.concatenate(
        [W, w_src.astype(np.float32), w_dst.astype(np.float32)], axis=1
    )  # [IN, 264]

    # table-row permutation: quarter-major so each pipelined AllGather
    # quarter writes a contiguous output block. bounds align to 128-row tiles.
    ntiles = (npc + P - 1) // P
    # single gather quarter measured faster than 4 pipelined ones
    qt = [0, npc]
    qb = np.array(qt, np.int64)

    def perm_row(nodes):
        nodes = np.asarray(nodes, np.int64)
        r = nodes // npc
        i = nodes % npc
        q = np.searchsorted(qb, i, side="right") - 1
        qlen = qb[q + 1] - qb[q]
        return ncores * qb[q] + r * qlen + (i - qb[q])

    owner = dst // npc
    # per (core, window): edge lists split into lo (src < half) / hi
    lists = [[None] * nw for _ in range(ncores)]
    for c in range(ncores):
        ids = np.nonzero(owner == c)[0]
        dl = (dst[ids] - c * npc).astype(np.int64)
        order = np.argsort(dl, kind="stable")
        ids = ids[order]
        dl = dl[order]
        w_of = dl // P
        for w in range(nw):
            sel = w_of == w
            eids = ids[sel]
            edl = dl[sel]
            prow = perm_row(src[eids])
            lo = prow < half
            lists[c][w] = (
                (eids[lo], edl[lo], prow[lo]),
                (eids[~lo], edl[~lo], prow[~lo] - half),
            )

    kl = [
        max(1, max(-(-len(lists[c][w][0][0]) // P) for c in range(ncores)))
        for w in range(nw)
    ]
    kh = [
        max(1, max(-(-len(lists[c][w][1][0]) // P) for c in range(ncores)))
        for w in range(nw)
    ]
    kw = [kl[w] + kh[w] for w in range(nw)]
    ctot = int(sum(kw))
    col0 = np.cumsum([0] + kw)

    idx16 = np.zeros((ncores, P, 8 * ctot), np.int16)
    dstrel = np.full((ncores, P, ctot), PAD_DST, np.float32)
    att_pos = []
    for c in range(ncores):
        pos_all = []
        ids_all = []
        for w in range(nw):
            (eL, dL, sL), (eH, dH, sH) = lists[c][w]
            for (ee, dd, ss, kk, cbase) in (
                (eL, dL, sL, kl[w], col0[w]),
                (eH, dH, sH, kh[w], col0[w] + kl[w]),
            ):
                npad = kk * P
                svals = np.zeros(npad, np.int64)
                svals[: len(ss)] = ss
                idx16[c, :, 8 * cbase : 8 * (cbase + kk)] = _wrap16(svals)
                pp = cbase * P + np.arange(len(ee))
                pos_all.append(pp)
                ids_all.append(ee)
                dstrel[c, pp % P, pp // P] = (dd - w * P).astype(np.float32)
        att_pos.append((np.concatenate(ids_all), np.concatenate(pos_all)))

    iota_col = np.arange(P, dtype=np.float32).astype(np.float32)
    in_maps = []
    for c in range(ncores):
        xt = np.ascontiguousarray(x[c * npc : (c + 1) * npc].T)
        in_maps.append(
            {
                "xt": xt,
                "wext": wext,
                "idx16": idx16[c],
                "dstrel_h": dstrel[c].astype(np.dtype("bfloat16"))
                if hasattr(np, "bfloat16")
                else dstrel[c],  # replaced below
                "identity_h": None,
            }
        )
    # bf16 via ml_dtypes
    import ml_dtypes

    bf = ml_dtypes.bfloat16
    iota_rep = np.broadcast_to(np.arange(P, dtype=np.float32), (P, P))
    identity = np.eye(P, dtype=np.float32)
    for c in range(ncores):
        in_maps[c]["dstrel_h"] = dstrel[c].astype(bf)
        in_maps[c]["dstrel32"] = dstrel[c]
        in_maps[c]["iota_col32"] = iota_col.reshape(P, 1)
        in_maps[c]["iota_rep32"] = np.ascontiguousarray(iota_rep)
        in_maps[c]["identity_h"] = identity.astype(bf)

    meta = {
        "qt": tuple(int(v) for v in qt),
        "n": n,
        "ncores": ncores,
        "npc": npc,
        "nw": nw,
        "kl": tuple(kl),
        "kh": tuple(kh),
        "kw": tuple(kw),
        "ctot": ctot,
        "col0": tuple(int(v) for v in col0),
    }
    return in_maps, meta, att_pos


# ---------------------------------------------------------------- kernel gen
def _build(meta, linearize=False):
    npc = meta["npc"]
    nw = meta["nw"]
    kl, kh, kw = meta["kl"], meta["kh"], meta["kw"]
    ctot = meta["ctot"]
    col0 = meta["col0"]
    ncores = meta["ncores"]
    n = meta["n"]
    half = n // 2
    maxk = max(kw)
    D_AH = 264  # phase-A psum width: xh | a_src | a_dst

    nc = bacc.Bacc(
        "TRN2", target_bir_lowering=False, debug=False, num_devices=ncores
    )

    xt = nc.declare_dram_parameter("xt", [IN, npc], F32, isOutput=False)
    wext = nc.declare_dram_parameter("wext", [IN, D_AH], F32, isOutput=False)
    idx16_d = nc.declare_dram_parameter("idx16", [P, 8 * ctot], I16, isOutput=False)
    dstrel_d = nc.declare_dram_parameter("dstrel_h", [P, ctot], BF16, isOutput=False)
    dstrel32_d = nc.declare_dram_parameter("dstrel32", [P, ctot], F32, isOutput=False)
    iota_col_d = nc.declare_dram_parameter("iota_col32", [P, 1], F32, isOutput=False)
    iota_rep_d = nc.declare_dram_parameter("iota_rep32", [P, P], F32, isOutput=False)
    identity_d = nc.declare_dram_parameter("identity_h", [P, P], BF16, isOutput=False)

    out_d = nc.declare_dram_parameter("out", [npc, D_XH], F32, isOutput=True)
    att_d = nc.declare_dram_parameter("att", [ctot * P, H], F32, isOutput=True)

    xh_local = nc.dram_tensor("xh_local", [npc, TW], BF16)
    xh_table = nc.dram_tensor("xh_table", [n, TW], BF16, addr_space="Shared")

    a_dst_sb = nc.alloc_sbuf_tensor("a_dst_sb", [P, H * nw], F32)
    idx16_sb = nc.alloc_sbuf_tensor("idx16_sb", [P, 8 * ctot], I16)
    dstrel_sb = nc.alloc_sbuf_tensor("dstrel_sb", [P, ctot], BF16)
    dstrel32_sb = nc.alloc_sbuf_tensor("dstrel32_sb", [P, ctot], F32)
    iota_col_sb = nc.alloc_sbuf_tensor("iota_col_sb", [P, 1], F32)
    iota_rep_sb = nc.alloc_sbuf_tensor("iota_rep_sb", [P, P], F32)
    identity_sb = nc.alloc_sbuf_tensor("identity_sb", [P, P], BF16)
    adhl_sb = nc.alloc_sbuf_tensor("adhl_sb", [P, nw * 2 * H], BF16)

    ntiles = (npc + P - 1) // P
    qt = meta["qt"]

    # Q7 library with the dma_gather kernel; must precede any dma_gather.
    nc.gpsimd.load_library(mlp)
    from concourse.tile import add_dep_helper

    # ---------------- phase A ------------------------------------------
    with tile.TileContext(nc, linearize=linearize) as tc:
        with (
            tc.tile_pool(name="xa", bufs=1) as xa,
            tc.tile_pool(name="wa", bufs=1) as wa,
            tc.tile_pool(name="sba", bufs=3) as sba,
            tc.tile_pool(name="psa", bufs=2, space="PSUM") as psa,
        ):
            xt_sb = [
                xa.tile([P, npc], F32, tag=f"xt{i}", name=f"xt_sb{i}")
                for i in range(2)
            ]
            w_sb = [
                wa.tile([P, D_AH], F32, tag=f"w{i}", name=f"w_sb{i}")
                for i in range(2)
            ]
            for i in range(2):
                nc.sync.dma_start(out=xt_sb[i][:], in_=xt[i * P : (i + 1) * P, :])
                nc.sync.dma_start(out=w_sb[i][:], in_=wext[i * P : (i + 1) * P, :])
            nc.sync.dma_start(out=idx16_sb[:, :], in_=idx16_d[:, :])
            nc.sync.dma_start(out=dstrel_sb[:, :], in_=dstrel_d[:, :])
            nc.sync.dma_start(out=dstrel32_sb[:, :], in_=dstrel32_d[:, :])
            nc.sync.dma_start(out=iota_col_sb[:, :], in_=iota_col_d[:, :])
            nc.sync.dma_start(out=iota_rep_sb[:, :], in_=iota_rep_d[:, :])
            nc.sync.dma_start(out=identity_sb[:, :], in_=identity_d[:, :])

            nc.vector.memset(a_dst_sb[:, :], 0)
            row_dmas = []
            for t in range(ntiles):
                rows = min(P, npc - t * P)
                ps = psa.tile([P, D_AH], F32, space="PSUM", tag="psA")
                nc.tensor.matmul(
                    out=ps[:rows, :],
                    lhsT=xt_sb[0][:, t * P : t * P + rows],
                    rhs=w_sb[0][:],
                    start=True,
                    stop=False,
                )
                nc.tensor.matmul(
                    out=ps[:rows, :],
                    lhsT=xt_sb[1][:, t * P : t * P + rows],
                    rhs=w_sb[1][:],
                    start=False,
                    stop=True,
                )
                row = sba.tile([P, TW], BF16, tag="rowA")
                nc.vector.tensor_copy(
                    out=row[:rows, 0:D_XH], in_=ps[:rows, 0:D_XH]
                )
                # a_src, a_dst kept exact: f32 bits stored in bf16 columns
                nc.vector.tensor_copy(
                    out=row[:rows, D_XH : D_XH + 16].bitcast(F32),
                    in_=ps[:rows, D_XH:D_AH],
                )
                nc.vector.tensor_copy(
                    out=a_dst_sb[:rows, H * t : H * t + H],
                    in_=ps[:rows, D_XH + H : D_AH],
                )
                row_dmas.append(
                    nc.sync.dma_start(
                        out=xh_local[t * P : t * P + rows, :], in_=row[:rows, :]
                    )
                )
                qhit = [qi for qi in range(len(qt) - 1) if qt[qi + 1] == t * P + rows]
                if qhit:
                    qi = qhit[0]
                    r0, r1 = qt[qi], qt[qi + 1]
                    cc = nc.gpsimd.collective_compute(
                        "AllGather",
                        mybir.AluOpType.bypass,
                        replica_groups=[list(range(ncores))],
                        ins=[xh_local[r0:r1, :].opt()],
                        outs=[xh_table[ncores * r0 : ncores * r1, :].opt()],
                    )
                    for dm in row_dmas:
                        add_dep_helper(
                            cc.ins, dm.ins, sync=True,
                            reason="AllGather quarter waits on table-row DMAs",
                        )
                    row_dmas = []
            # a_dst bf16 hi/lo pairs for exact bf16-matmul gathers
            ahi32 = sba.tile([P, H * nw], F32, tag="ahi32", name="ahi32")
            adhl = adhl_sb[:, :].rearrange("p (w u h) -> p w u h", u=2, h=H)
            nc.vector.tensor_copy(
                out=adhl[:, :, 0, :],
                in_=a_dst_sb[:, :].rearrange("p (w h) -> p w h", h=H),
            )
            nc.vector.tensor_copy(
                out=ahi32[:].rearrange("p (w h) -> p w h", h=H), in_=adhl[:, :, 0, :]
            )
            nc.vector.tensor_sub(out=ahi32[:], in0=a_dst_sb[:, :], in1=ahi32[:])
            nc.vector.tensor_copy(
                out=adhl[:, :, 1, :],
                in_=ahi32[:].rearrange("p (w h) -> p w h", h=H),
            )

    # ---------------- phase C ------------------------------------------
    t_lo = xh_table[0:half, :]
    t_hi = xh_table[half:n, :]
    with tile.TileContext(nc, linearize=linearize) as tc:
        with (
            tc.tile_pool(name="gat", bufs=3) as gp,
            tc.tile_pool(name="rhs", bufs=3) as rp,
            tc.tile_pool(name="amat", bufs=2 * maxk + 4) as ap_,
            tc.tile_pool(name="atmat", bufs=2 * ((maxk + 3) // 4) + 2) as atp,
            tc.tile_pool(name="small", bufs=3) as sp,
            tc.tile_pool(name="winps", bufs=3, space="PSUM") as wpp,
            tc.tile_pool(name="dstps", bufs=2, space="PSUM") as dpp,
            tc.tile_pool(name="smps", bufs=2, space="PSUM") as spp,
            tc.tile_pool(name="exw", bufs=3) as exp_,
            tc.tile_pool(name="outw", bufs=3) as op_,
        ):
            for w in range(nw):
                k = kw[w]
                g0 = col0[w]
                rows = min(P, npc - w * P)
                gatb = gp.tile([P, k, TW], BF16, tag="gat")
                # dma_gather crashes the device above ~1024 idxs per call
                for base, cnt, tbl in (
                    (0, kl[w], t_lo),
                    (kl[w], kh[w], t_hi),
                ):
                    off = 0
                    while off < cnt:
                        m = min(8, cnt - off)
                        c0 = g0 + base + off
                        nc.gpsimd.dma_gather(
                            gatb[:, base + off : base + off + m, :],
                            tbl,
                            idx16_sb[:, 8 * c0 : 8 * (c0 + m)],
                            m * P,
                            m * P,
                            TW,
                        )
                        off += m
                # one-hot builds + transposes + a_dst gather matmuls
                amats = []
                atbs = []
                adstb = spp.tile([P, k, 2 * H], F32, space="PSUM", tag="smps")
                for j in range(k):
                    g = g0 + j
                    jm = j % 4
                    if jm == 0:
                        nb = min(4, k - j)
                        dstTb = dpp.tile([P, 4 * P], BF16, space="PSUM", tag="dstT")
                        atb = atp.tile([P, 4 * P], BF16, tag="at")
                    nc.tensor.transpose(
                        out=dstTb[:, jm * P : (jm + 1) * P],
                        in_=dstrel_sb[:, g : g + 1].to_broadcast([P, P]),
                        identity=identity_sb[:],
                    )
                    if jm == nb - 1:
                        nc.vector.tensor_tensor(
                            out=atb[:, 0 : nb * P],
                            in0=iota_col_sb[:, 0:1].to_broadcast([P, nb * P]),
                            in1=dstTb[:, 0 : nb * P],
                            op=mybir.AluOpType.is_equal,
                        )
                        atbs.append(atb)
                    amat = ap_.tile([P, P], BF16, tag="amat")
                    nc.vector.tensor_tensor(
                        out=amat[:],
                        in0=dstrel32_sb[:, g : g + 1].to_broadcast([P, P]),
                        in1=iota_rep_sb[:],
                        op=mybir.AluOpType.is_equal,
                    )
                    amats.append(amat)
                for j in range(k):
                    nc.tensor.matmul(
                        out=adstb[:, j, :],
                        lhsT=atbs[j // 4][:, (j % 4) * P : (j % 4 + 1) * P],
                        rhs=adhl_sb[:, 2 * H * w : 2 * H * (w + 1)],
                        start=True,
                        stop=True,
                    )
                # batched alpha -> ex -> rhs
                exw = exp_.tile([P, k, H], F32, tag="exw")
                asrc = gatb[:, :, D_XH : D_XH + 8].bitcast(F32)  # [P,k,4]
                nc.vector.tensor_add(
                    out=exw[:], in0=asrc, in1=adstb[:, :, 0:H]
                )
                nc.vector.tensor_add(out=exw[:], in0=exw[:], in1=adstb[:, :, H:])
                lk = sp.tile([P, k, H], F32, tag="lk")
                nc.vector.tensor_scalar_mul(out=lk[:], in0=exw[:], scalar1=NEG_SLOPE)
                nc.vector.tensor_tensor(
                    out=exw[:], in0=exw[:], in1=lk[:], op=mybir.AluOpType.max
                )
                nc.scalar.activation(
                    out=exw[:], in_=exw[:], func=mybir.ActivationFunctionType.Exp
                )
                rhsb = rp.tile([P, k, D_EXT], BF16, tag="rhs")
                nc.scalar.copy(out=rhsb[:, :, D_XH:D_EXT], in_=exw[:])
                nc.vector.tensor_tensor(
                    out=rhsb[:, :, 0:D_XH].rearrange("p k (h c) -> p k h c", h=H),
                    in0=gatb[:, :, 0:D_XH].rearrange("p k (h c) -> p k h c", h=H),
                    in1=exw[:].to_broadcast([P, k, H, C]),
                    op=mybir.AluOpType.mult,
                )
                # scatter-accumulate
                ps = wpp.tile([P, D_EXT], F32, space="PSUM", tag="win")
                for j in range(k):
                    nc.tensor.matmul(
                        out=ps[:],
                        lhsT=amats[j][:],
                        rhs=rhsb[:, j, :],
                        start=(j == 0),
                        stop=(j == k - 1),
                    )
                # normalize
                den = sp.tile([P, H], F32, tag="den")
                nc.vector.tensor_scalar_add(
                    out=den[:], in0=ps[:, D_XH:D_EXT], scalar1=EPS
                )
                denrec = sp.tile([P, H], F32, tag="denrec")
                nc.vector.reciprocal(out=denrec[:], in_=den[:])
                outsb = op_.tile([P, D_XH], F32, tag="outw")
                nc.vector.tensor_tensor(
                    out=outsb[:].rearrange("p (h c) -> p h c", h=H),
                    in0=ps[:, 0:D_XH].rearrange("p (h c) -> p h c", h=H),
                    in1=denrec[:].to_broadcast([P, H, C]),
                    op=mybir.AluOpType.mult,
                )
                nc.sync.dma_start(
                    out=out_d[w * P : w * P + rows, :], in_=outsb[:rows, :]
                )
                # denrec hi/lo for exact per-edge gather
                drhl = sp.tile([P, 2 * H], BF16, tag="drhl")
                dr32 = sp.tile([P, H], F32, tag="dr32")
                nc.scalar.copy(out=drhl[:, 0:H], in_=denrec[:])
                nc.scalar.copy(out=dr32[:], in_=drhl[:, 0:H])
                nc.vector.tensor_sub(out=dr32[:], in0=denrec[:], in1=dr32[:])
                nc.scalar.copy(out=drhl[:, H:], in_=dr32[:])
                attb = spp.tile([P, k, 2 * H], F32, space="PSUM", tag="smps")
                for j in range(k):
                    nc.tensor.matmul(
                        out=attb[:, j, :],
                        lhsT=atbs[j // 4][:, (j % 4) * P : (j % 4 + 1) * P],
                        rhs=drhl[:],
                        start=True,
                        stop=True,
                    )
                attw = exp_.tile([P, k, H], F32, tag="attw")
                nc.scalar.copy(out=attw[:], in_=attb[:, :, 0:H])
                nc.vector.tensor_add(out=attw[:], in0=attw[:], in1=attb[:, :, H:])
                nc.vector.tensor_tensor(
                    out=attw[:], in0=attw[:], in1=exw[:], op=mybir.AluOpType.mult
                )
                att_view = att_d[g0 * P : (g0 + k) * P, :].rearrange(
                    "(k p) h -> p k h", p=P
                )
                nc.sync.dma_start(out=att_view, in_=attw[:])

    nc.compile()
    return nc


_CACHE = {}


def _get_kernel(meta):
    key = (meta["n"], meta["ncores"], meta["kl"], meta["kh"])
    if key not in _CACHE:
        _CACHE[key] = _build(meta)
    return _CACHE[key]


def kernel(x, edge_index, W, att_src, att_dst):
    in_maps, meta, att_pos = _prep(x, edge_index, W, att_src, att_dst)
    nc = _get_kernel(meta)
    res = run_bass_kernel_spmd(nc, in_maps, core_ids=list(range(meta["ncores"])))
    ncores = meta["ncores"]
    out = np.concatenate([res.results[c]["out"] for c in range(ncores)], axis=0)
    att = np.empty((E, H), np.float32)
    for c in range(ncores):
        ids, pos = att_pos[c]
        att[ids] = res.results[c]["att"][pos]
    return out, att


# revision 19
# speedup vs baseline: 1.0916x; 1.0043x over previous
"""GAT layer (4-head, 256->4x64) on 8 Trainium2 NeuronCores.

Strategy (1D node partitioning per the sharding hint):
  - core c owns dst nodes [c*6250, (c+1)*6250) and every edge whose dst
    falls in that range (host buckets edges; sorts by dst).
  - phase A: each core projects its own node block: xh_ext = x_c @ [W | W@att_src | W@att_dst]
    producing bf16 table rows [xh bf16 (256) | a_src f32 (as 8 bf16) |
    a_dst f32 (as 8 bf16) | pad] of 768B, plus an SBUF copy of local a_dst.
  - phase B: AllGather of the table (the "halo exchange" -- with a random
    graph every core needs nearly all rows).
  - phase C: per 128-dst window: dma_gather of the window's edges' rows by
    src id (two calls: table halves, since dma_gather indices are int16),
    one-hot matmul scatter-add into a PSUM window accumulator of
    [sum_e ex_e*xh_e | sum_e ex_e]; normalize after accumulation (softmax
    denominator division commutes with the sum). Segment max is skipped:
    logits are bounded (|alpha| <~ 20) so raw exp is safe in fp32, and the
    reference's +1e-16 epsilon is preserved.
  - per-edge attention: att = ex * (1/denom)[dst], with dst-indexed gathers
    done as one-hot matmuls (lhsT = one-hot^T); a_dst and 1/denom are fed
    as bf16 hi/lo pairs so those gathers stay fp32-exact.
  - att output is produced in sorted-edge order; the host applies the
    inverse permutation.
"""

import os
import sys
import types

if os.environ.get("JAX_PLATFORMS") == "cpu":
    # the PJRT execute path needs the neuron/axon backend
    del os.environ["JAX_PLATFORMS"]

import numpy as np

import concourse.bass as bass
import concourse.mybir as mybir
import concourse.tile as tile
from concourse import bacc
from concourse.bass_utils import run_bass_kernel_spmd
from concourse.library_config import mlp

N = 50000
E = 800000
IN = 256
H = 4
C = 64
NEG_SLOPE = 0.2
EPS = 1e-16
NCORES = 8
P = 128
NPC = N // NCORES          # nodes per core
NW = (NPC + P - 1) // P    # dst windows per core
D_XH = H * C               # 256
D_EXT = D_XH + H           # 260 : xh | ex  (scatter-matmul rhs width)
TW = 384                   # bf16 table row: xh(256) | a_src f32(8) | a_dst f32(8) | pad
PAD_DST = 999.0

F32 = mybir.dt.float32
BF16 = mybir.dt.bfloat16
I16 = mybir.dt.int16
I32 = mybir.dt.int32


def _install_ntff_hook_shim():
    try:
        import antenv.axon_hooks  # noqa: F401
        return
    except ImportError:
        pass
    try:
        from trn_agent_boot.trn_boot import _ntff_profile_via_ctypes

        so = "/opt/axon/libaxon_pjrt.so"
        if not os.path.exists(so):
            return
        hook = _ntff_profile_via_ctypes(so)
        mod = types.ModuleType("antenv.axon_hooks")
        mod.get_axon_ntff_profile_hook = lambda: hook
        import antenv

        antenv.axon_hooks = mod
        sys.modules["antenv.axon_hooks"] = mod
    except Exception:
        pass


_install_ntff_hook_shim()


def _wrap16(vals):
    """int16 index list (len n, n%128==0) -> [128, n//16] dma_gather layout:
    logical index i sits at [i % 16, i // 16], replicated over the 8
    16-partition Q7 groups."""
    n = len(vals)
    arr = np.zeros((16, n // 16), np.int16)
    arr[np.arange(n) % 16, np.arange(n) // 16] = vals
    return np.tile(arr, (8, 1))


# ---------------------------------------------------------------- host prep
def _prep(x, edge_index, W, att_src, att_dst, n=N, ncores=NCORES):
    npc = n // ncores
    nw = (npc + P - 1) // P
    half = n // 2
    x = np.asarray(x, np.float32)
    W = np.asarray(W, np.float32)
    att_src = np.asarray(att_src, np.float32)
    att_dst = np.asarray(att_dst, np.float32)
    src = np.asarray(edge_index[0], np.int64)
    dst = np.asarray(edge_index[1], np.int64)

    W3 = W.reshape(IN, H, C)
    w_src = np.einsum("khc,hc->kh", W3.astype(np.float64), att_src.astype(np.float64))
    w_dst = np.einsum("khc,hc->kh", W3.astype(np.float64), att_dst.astype(np.float64))
    wext = np.concatenate(
        [W, w_src.astype(np.float32), w_dst.astype(np.float32)], axis=1
    )  # [IN, 264]

    # table-row permutation: quarter-major so each pipelined AllGather
    # quarter writes a contiguous output block. bounds align to 128-row tiles.
    ntiles = (npc + P - 1) // P
    # single gather quarter measured faster than 4 pipelined ones
    qt = [0, npc]
    qb = np.array(qt, np.int64)

    def perm_row(nodes):
        nodes = np.asarray(nodes, np.int64)
        r = nodes // npc
        i = nodes % npc
        q = np.searchsorted(qb, i, side="right") - 1
        qlen = qb[q + 1] - qb[q]
        return ncores * qb[q] + r * qlen + (i - qb[q])

    owner = dst // npc
    # per (core, window): edge lists split into lo (src < half) / hi
    lists = [[None] * nw for _ in range(ncores)]
    for c in range(ncores):
        ids = np.nonzero(owner == c)[0]
        dl = (dst[ids] - c * npc).astype(np.int64)
        order = np.argsort(dl, kind="stable")
        ids = ids[order]
        dl = dl[order]
        w_of = dl // P
        for w in range(nw):
            sel = w_of == w
            eids = ids[sel]
            edl = dl[sel]
            prow = perm_row(src[eids])
            lo = prow < half
            lists[c][w] = (
                (eids[lo], edl[lo], prow[lo]),
                (eids[~lo], edl[~lo], prow[~lo] - half),
            )

    kl = [
        max(1, max(-(-len(lists[c][w][0][0]) // P) for c in range(ncores)))
        for w in range(nw)
    ]
    kh = [
        max(1, max(-(-len(lists[c][w][1][0]) // P) for c in range(ncores)))
        for w in range(nw)
    ]
    kw = [kl[w] + kh[w] for w in range(nw)]
    ctot = int(sum(kw))
    col0 = np.cumsum([0] + kw)

    idx16 = np.zeros((ncores, P, 8 * ctot), np.int16)
    dstrel = np.full((ncores, P, ctot), PAD_DST, np.float32)
    att_pos = []
    for c in range(ncores):
        pos_all = []
        ids_all = []
        for w in range(nw):
            (eL, dL, sL), (eH, dH, sH) = lists[c][w]
            for (ee, dd, ss, kk, cbase) in (
                (eL, dL, sL, kl[w], col0[w]),
                (eH, dH, sH, kh[w], col0[w] + kl[w]),
            ):
                npad = kk * P
                svals = np.zeros(npad, np.int64)
                svals[: len(ss)] = ss
                idx16[c, :, 8 * cbase : 8 * (cbase + kk)] = _wrap16(svals)
                pp = cbase * P + np.arange(len(ee))
                pos_all.append(pp)
                ids_all.append(ee)
                dstrel[c, pp % P, pp // P] = (dd - w * P).astype(np.float32)
        att_pos.append((np.concatenate(ids_all), np.concatenate(pos_all)))

    iota_col = np.arange(P, dtype=np.float32).astype(np.float32)
    in_maps = []
    for c in range(ncores):
        xt = np.ascontiguousarray(x[c * npc : (c + 1) * npc].T)
        in_maps.append(
            {
                "xt": xt,
                "wext": wext,
                "idx16": idx16[c],
                "dstrel_h": dstrel[c].astype(np.dtype("bfloat16"))
                if hasattr(np, "bfloat16")
                else dstrel[c],  # replaced below
                "identity_h": None,
            }
        )
    # bf16 via ml_dtypes
    import ml_dtypes

    bf = ml_dtypes.bfloat16
    iota_rep = np.broadcast_to(np.arange(P, dtype=np.float32), (P, P))
    identity = np.eye(P, dtype=np.float32)
    for c in range(ncores):
        in_maps[c]["dstrel_h"] = dstrel[c].astype(bf)
        in_maps[c]["dstrel32"] = dstrel[c]
        in_maps[c]["iota_col32"] = iota_col.reshape(P, 1)
        in_maps[c]["iota_rep32"] = np.ascontiguousarray(iota_rep)
        in_maps[c]["identity_h"] = identity.astype(bf)

    meta = {
        "qt": tuple(int(v) for v in qt),
        "n": n,
        "ncores": ncores,
        "npc": npc,
        "nw": nw,
        "kl": tuple(kl),
        "kh": tuple(kh),
        "kw": tuple(kw),
        "ctot": ctot,
        "col0": tuple(int(v) for v in col0),
    }
    return in_maps, meta, att_pos


# ---------------------------------------------------------------- kernel gen
def _build(meta, linearize=False):
    npc = meta["npc"]
    nw = meta["nw"]
    kl, kh, kw = meta["kl"], meta["kh"], meta["kw"]
    ctot = meta["ctot"]
    col0 = meta["col0"]
    ncores = meta["ncores"]
    n = meta["n"]
    half = n // 2
    maxk = max(kw)
    D_AH = 264  # phase-A psum width: xh | a_src | a_dst

    nc = bacc.Bacc(
        "TRN2", target_bir_lowering=False, debug=False, num_devices=ncores
    )

    xt = nc.declare_dram_parameter("xt", [IN, npc], F32, isOutput=False)
    wext = nc.declare_dram_parameter("wext", [IN, D_AH], F32, isOutput=False)
    idx16_d = nc.declare_dram_parameter("idx16", [P, 8 * ctot], I16, isOutput=False)
    dstrel_d = nc.declare_dram_parameter("dstrel_h", [P, ctot], BF16, isOutput=False)
    dstrel32_d = nc.declare_dram_parameter("dstrel32", [P, ctot], F32, isOutput=False)
    iota_col_d = nc.declare_dram_parameter("iota_col32", [P, 1], F32, isOutput=False)
    iota_rep_d = nc.declare_dram_parameter("iota_rep32", [P, P], F32, isOutput=False)
    identity_d = nc.declare_dram_parameter("identity_h", [P, P], BF16, isOutput=False)

    out_d = nc.declare_dram_parameter("out", [npc, D_XH], F32, isOutput=True)
    att_d = nc.declare_dram_parameter("att", [ctot * P, H], F32, isOutput=True)

    xh_local = nc.dram_tensor("xh_local", [npc, TW], BF16)
    xh_table = nc.dram_tensor("xh_table", [n, TW], BF16, addr_space="Shared")

    a_dst_sb = nc.alloc_sbuf_tensor("a_dst_sb", [P, H * nw], F32)
    idx16_sb = nc.alloc_sbuf_tensor("idx16_sb", [P, 8 * ctot], I16)
    dstrel_sb = nc.alloc_sbuf_tensor("dstrel_sb", [P, ctot], BF16)
    dstrel32_sb = nc.alloc_sbuf_tensor("dstrel32_sb", [P, ctot], F32)
    iota_col_sb = nc.alloc_sbuf_tensor("iota_col_sb", [P, 1], F32)
    iota_rep_sb = nc.alloc_sbuf_tensor("iota_rep_sb", [P, P], F32)
    identity_sb = nc.alloc_sbuf_tensor("identity_sb", [P, P], BF16)
    adhl_sb = nc.alloc_sbuf_tensor("adhl_sb", [P, nw * 2 * H], BF16)

    ntiles = (npc + P - 1) // P
    qt = meta["qt"]

    # Q7 library with the dma_gather kernel; must precede any dma_gather.
    nc.gpsimd.load_library(mlp)
    from concourse.tile import add_dep_helper

    # ---------------- phase A ------------------------------------------
    with tile.TileContext(nc, linearize=linearize) as tc:
        with (
            tc.tile_pool(name="xa", bufs=1) as xa,
            tc.tile_pool(name="wa", bufs=1) as wa,
            tc.tile_pool(name="sba", bufs=3) as sba,
            tc.tile_pool(name="psa", bufs=2, space="PSUM") as psa,
        ):
            xt_sb = [
                xa.tile([P, npc], F32, tag=f"xt{i}", name=f"xt_sb{i}")
                for i in range(2)
            ]
            w_sb = [
                wa.tile([P, D_AH], F32, tag=f"w{i}", name=f"w_sb{i}")
                for i in range(2)
            ]
            for i in range(2):
                nc.sync.dma_start(out=xt_sb[i][:], in_=xt[i * P : (i + 1) * P, :])
                nc.sync.dma_start(out=w_sb[i][:], in_=wext[i * P : (i + 1) * P, :])
            nc.sync.dma_start(out=idx16_sb[:, :], in_=idx16_d[:, :])
            nc.sync.dma_start(out=dstrel_sb[:, :], in_=dstrel_d[:, :])
            nc.sync.dma_start(out=dstrel32_sb[:, :], in_=dstrel32_d[:, :])
            nc.sync.dma_start(out=iota_col_sb[:, :], in_=iota_col_d[:, :])
            nc.sync.dma_start(out=iota_rep_sb[:, :], in_=iota_rep_d[:, :])
            nc.sync.dma_start(out=identity_sb[:, :], in_=identity_d[:, :])

            nc.vector.memset(a_dst_sb[:, :], 0)
            row_dmas = []
            for t in range(ntiles):
                rows = min(P, npc - t * P)
                ps = psa.tile([P, D_AH], F32, space="PSUM", tag="psA")
                nc.tensor.matmul(
                    out=ps[:rows, :],
                    lhsT=xt_sb[0][:, t * P : t * P + rows],
                    rhs=w_sb[0][:],
                    start=True,
                    stop=False,
                )
                nc.tensor.matmul(
                    out=ps[:rows, :],
                    lhsT=xt_sb[1][:, t * P : t * P + rows],
                    rhs=w_sb[1][:],
                    start=False,
                    stop=True,
                )
                row = sba.tile([P, TW], BF16, tag="rowA")
                nc.vector.tensor_copy(
                    out=row[:rows, 0:D_XH], in_=ps[:rows, 0:D_XH]
                )
                # a_src, a_dst kept exact: f32 bits stored in bf16 columns
                nc.vector.tensor_copy(
                    out=row[:rows, D_XH : D_XH + 16].bitcast(F32),
                    in_=ps[:rows, D_XH:D_AH],
                )
                nc.vector.tensor_copy(
                    out=a_dst_sb[:rows, H * t : H * t + H],
                    in_=ps[:rows, D_XH + H : D_AH],
                )
                row_dmas.append(
                    nc.sync.dma_start(
                        out=xh_local[t * P : t * P + rows, :], in_=row[:rows, :]
                    )
                )
                qhit = [qi for qi in range(len(qt) - 1) if qt[qi + 1] == t * P + rows]
                if qhit:
                    qi = qhit[0]
                    r0, r1 = qt[qi], qt[qi + 1]
                    cc = nc.gpsimd.collective_compute(
                        "AllGather",
                        mybir.AluOpType.bypass,
                        replica_groups=[list(range(ncores))],
                        ins=[xh_local[r0:r1, :].opt()],
                        outs=[xh_table[ncores * r0 : ncores * r1, :].opt()],
                    )
                    for dm in row_dmas:
                        add_dep_helper(
                            cc.ins, dm.ins, sync=True,
                            reason="AllGather quarter waits on table-row DMAs",
                        )
                    row_dmas = []
            # a_dst bf16 hi/lo pairs for exact bf16-matmul gathers
            ahi32 = sba.tile([P, H * nw], F32, tag="ahi32", name="ahi32")
            adhl = adhl_sb[:, :].rearrange("p (w u h) -> p w u h", u=2, h=H)
            nc.vector.tensor_copy(
                out=adhl[:, :, 0, :],
                in_=a_dst_sb[:, :].rearrange("p (w h) -> p w h", h=H),
            )
            nc.vector.tensor_copy(
                out=ahi32[:].rearrange("p (w h) -> p w h", h=H), in_=adhl[:, :, 0, :]
            )
            nc.vector.tensor_sub(out=ahi32[:], in0=a_dst_sb[:, :], in1=ahi32[:])
            nc.vector.tensor_copy(
                out=adhl[:, :, 1, :],
                in_=ahi32[:].rearrange("p (w h) -> p w h", h=H),
            )

    # ---------------- phase C ------------------------------------------
    t_lo = xh_table[0:half, :]
    t_hi = xh_table[half:n, :]
    with tile.TileContext(nc, linearize=linearize) as tc:
        with (
            tc.tile_pool(name="gat", bufs=4) as gp,
            tc.tile_pool(name="rhs", bufs=3) as rp,
            tc.tile_pool(name="amat", bufs=2 * maxk + 4) as ap_,
            tc.tile_pool(name="atmat", bufs=3 * ((maxk + 3) // 4) + 2) as atp,
            tc.tile_pool(name="small", bufs=3) as sp,
            tc.tile_pool(name="winps", bufs=3, space="PSUM") as wpp,
            tc.tile_pool(name="dstps", bufs=2, space="PSUM") as dpp,
            tc.tile_pool(name="smps", bufs=2, space="PSUM") as spp,
            tc.tile_pool(name="exw", bufs=3) as exp_,
            tc.tile_pool(name="outw", bufs=3) as op_,
        ):
            for w in range(nw):
                k = kw[w]
                g0 = col0[w]
                rows = min(P, npc - w * P)
                gatb = gp.tile([P, k, TW], BF16, tag="gat")
                # dma_gather crashes the device above ~1024 idxs per call
                for base, cnt, tbl in (
                    (0, kl[w], t_lo),
                    (kl[w], kh[w], t_hi),
                ):
                    off = 0
                    while off < cnt:
                        m = min(8, cnt - off)
                        c0 = g0 + base + off
                        nc.gpsimd.dma_gather(
                            gatb[:, base + off : base + off + m, :],
                            tbl,
                            idx16_sb[:, 8 * c0 : 8 * (c0 + m)],
                            m * P,
                            m * P,
                            TW,
                        )
                        off += m
                # one-hot builds + transposes + a_dst gather matmuls
                amats = []
                atbs = []
                adstb = spp.tile([P, k, 2 * H], F32, space="PSUM", tag="smps")
                for j in range(k):
                    g = g0 + j
                    jm = j % 4
                    if jm == 0:
                        nb = min(4, k - j)
                        dstTb = dpp.tile([P, 4 * P], BF16, space="PSUM", tag="dstT")
                        atb = atp.tile([P, 4 * P], BF16, tag="at")
                    nc.tensor.transpose(
                        out=dstTb[:, jm * P : (jm + 1) * P],
                        in_=dstrel_sb[:, g : g + 1].to_broadcast([P, P]),
                        identity=identity_sb[:],
                    )
                    if jm == nb - 1:
                        nc.vector.tensor_tensor(
                            out=atb[:, 0 : nb * P],
                            in0=iota_col_sb[:, 0:1].to_broadcast([P, nb * P]),
                            in1=dstTb[:, 0 : nb * P],
                            op=mybir.AluOpType.is_equal,
                        )
                        atbs.append(atb)
                    amat = ap_.tile([P, P], BF16, tag="amat")
                    nc.vector.tensor_tensor(
                        out=amat[:],
                        in0=dstrel32_sb[:, g : g + 1].to_broadcast([P, P]),
                        in1=iota_rep_sb[:],
                        op=mybir.AluOpType.is_equal,
                    )
                    amats.append(amat)
                for j in range(k):
                    nc.tensor.matmul(
                        out=adstb[:, j, :],
                        lhsT=atbs[j // 4][:, (j % 4) * P : (j % 4 + 1) * P],
                        rhs=adhl_sb[:, 2 * H * w : 2 * H * (w + 1)],
                        start=True,
                        stop=True,
                    )
                # batched alpha -> ex -> rhs
                exw = exp_.tile([P, k, H], F32, tag="exw")
                asrc = gatb[:, :, D_XH : D_XH + 8].bitcast(F32)  # [P,k,4]
                nc.vector.tensor_add(
                    out=exw[:], in0=asrc, in1=adstb[:, :, 0:H]
                )
                nc.vector.tensor_add(out=exw[:], in0=exw[:], in1=adstb[:, :, H:])
                lk = sp.tile([P, k, H], F32, tag="lk")
                nc.vector.tensor_scalar_mul(out=lk[:], in0=exw[:], scalar1=NEG_SLOPE)
                nc.vector.tensor_tensor(
                    out=exw[:], in0=exw[:], in1=lk[:], op=mybir.AluOpType.max
                )
                nc.scalar.activation(
                    out=exw[:], in_=exw[:], func=mybir.ActivationFunctionType.Exp
                )
                rhsb = rp.tile([P, k, D_EXT], BF16, tag="rhs")
                nc.scalar.copy(out=rhsb[:, :, D_XH:D_EXT], in_=exw[:])
                nc.vector.tensor_tensor(
                    out=rhsb[:, :, 0:D_XH].rearrange("p k (h c) -> p k h c", h=H),
                    in0=gatb[:, :, 0:D_XH].rearrange("p k (h c) -> p k h c", h=H),
                    in1=exw[:].to_broadcast([P, k, H, C]),
                    op=mybir.AluOpType.mult,
                )
                # scatter-accumulate
                ps = wpp.tile([P, D_EXT], F32, space="PSUM", tag="win")
                for j in range(k):
                    nc.tensor.matmul(
                        out=ps[:],
                        lhsT=amats[j][:],
                        rhs=rhsb[:, j, :],
                        start=(j == 0),
                        stop=(j == k - 1),
                    )
                # normalize
                den = sp.tile([P, H], F32, tag="den")
                nc.vector.tensor_scalar_add(
                    out=den[:], in0=ps[:, D_XH:D_EXT], scalar1=EPS
                )
                denrec = sp.tile([P, H], F32, tag="denrec")
                nc.vector.reciprocal(out=denrec[:], in_=den[:])
                outsb = op_.tile([P, D_XH], F32, tag="outw")
                nc.vector.tensor_tensor(
                    out=outsb[:].rearrange("p (h c) -> p h c", h=H),
                    in0=ps[:, 0:D_XH].rearrange("p (h c) -> p h c", h=H),
                    in1=denrec[:].to_broadcast([P, H, C]),
                    op=mybir.AluOpType.mult,
                )
                nc.sync.dma_start(
                    out=out_d[w * P : w * P + rows, :], in_=outsb[:rows, :]
                )
                # denrec hi/lo for exact per-edge gather
                drhl = sp.tile([P, 2 * H], BF16, tag="drhl")
                dr32 = sp.tile([P, H], F32, tag="dr32")
                nc.scalar.copy(out=drhl[:, 0:H], in_=denrec[:])
                nc.scalar.copy(out=dr32[:], in_=drhl[:, 0:H])
                nc.vector.tensor_sub(out=dr32[:], in0=denrec[:], in1=dr32[:])
                nc.scalar.copy(out=drhl[:, H:], in_=dr32[:])
                attb = spp.tile([P, k, 2 * H], F32, space="PSUM", tag="smps")
                for j in range(k):
                    nc.tensor.matmul(
                        out=attb[:, j, :],
                        lhsT=atbs[j // 4][:, (j % 4) * P : (j % 4 + 1) * P],
                        rhs=drhl[:],
                        start=True,
                        stop=True,
                    )
                attw = exp_.tile([P, k, H], F32, tag="attw")
                nc.scalar.copy(out=attw[:], in_=attb[:, :, 0:H])
                nc.vector.tensor_add(out=attw[:], in0=attw[:], in1=attb[:, :, H:])
                nc.vector.tensor_tensor(
                    out=attw[:], in0=attw[:], in1=exw[:], op=mybir.AluOpType.mult
                )
                att_view = att_d[g0 * P : (g0 + k) * P, :].rearrange(
                    "(k p) h -> p k h", p=P
                )
                nc.sync.dma_start(out=att_view, in_=attw[:])

    nc.compile()
    return nc


_CACHE = {}


def _get_kernel(meta):
    key = (meta["n"], meta["ncores"], meta["kl"], meta["kh"])
    if key not in _CACHE:
        _CACHE[key] = _build(meta)
    return _CACHE[key]


def kernel(x, edge_index, W, att_src, att_dst):
    in_maps, meta, att_pos = _prep(x, edge_index, W, att_src, att_dst)
    nc = _get_kernel(meta)
    res = run_bass_kernel_spmd(nc, in_maps, core_ids=list(range(meta["ncores"])))
    ncores = meta["ncores"]
    out = np.concatenate([res.results[c]["out"] for c in range(ncores)], axis=0)
    att = np.empty((E, H), np.float32)
    for c in range(ncores):
        ids, pos = att_pos[c]
        att[ids] = res.results[c]["att"][pos]
    return out, att
